# revision 10
# baseline (speedup 1.0000x reference)
"""Trainium2 Bass kernel for EnhancedTMDO.

Computes, for x [B, S, F] and weight_matrix [F, F]:
  tmdo = 0.5 * time_diff + 0.5 * (x - x @ softmax(w, axis=1).T)
  lap  = 3x3 Laplacian-style conv over the (S, F) plane, SAME zero padding

Strategy (8 NeuronCores, data-parallel over batch, 4 batches per core):
  * All device compute happens in transposed layout [F=128 partitions, S free].
    The host pre-transposes each core's shard ([4,2048,128] -> [4,128,2048])
    and post-transposes the outputs back. This puts the feature contraction
    (the 128x128 matmuls) on the partition axis, and turns all seq-direction
    stencils into cheap shifted-AP reads along the free axis.
  * The 3x3 conv kernel is separable: outer([1,-2,1],[1,-2,1]). The seq-dim
    1D conv is computed from shifted adds; the feature-dim 1D conv is a
    tridiagonal 128x128 matmul on the TensorEngine.
  * Per 512-column chunk:
      psum_t = (-0.5 I - 0.5 w_sm.T).T @ x          (TensorE)
      sh     = x[s-1] + x[s+1]                      (VectorE, shifted APs)
      psum_l = A @ sh + (-2A) @ x                   (TensorE, accumulating;
                                                     A = tridiag(1,-2,1))
      tmdo   = 0.5*sh + psum_t                      (VectorE fused op)
      lap    = copy(psum_l)                         (ScalarE)
    plus a 1-column fixup per batch edge (time_diff is zero at s boundaries
    while the conv uses zero padding).
"""

from contextlib import ExitStack

import numpy as np

N_CORES = 8
B, S, F = 32, 2048, 128
B_PER = B // N_CORES
CHUNK = 512

_NC_CACHE = {}


def _build_nc(b_per=B_PER, s=S, chunk=CHUNK):
    import concourse.bacc as bacc
    import concourse.tile as tile
    from concourse import mybir

    f32 = mybir.dt.float32
    f32r = mybir.dt.float32r
    Alu = mybir.AluOpType
    Act = mybir.ActivationFunctionType

    nc = bacc.Bacc(None, target_bir_lowering=False)

    xt = nc.declare_dram_parameter("xt", [b_per, F, s], f32, isOutput=False)
    w = nc.declare_dram_parameter("w", [F, F], f32, isOutput=False)
    tmdo_d = nc.declare_dram_parameter("tmdo_t", [b_per, F, s], f32, isOutput=True)
    lap_d = nc.declare_dram_parameter("lap_t", [b_per, F, s], f32, isOutput=True)

    ident_np = np.eye(F, dtype=np.float32)
    a_np = (
        np.diag(np.full(F, -2.0))
        + np.diag(np.ones(F - 1), 1)
        + np.diag(np.ones(F - 1), -1)
    ).astype(np.float32)
    ident_dr = nc.inline_tensor(ident_np, "ident")
    a_dr = nc.inline_tensor(a_np, "amat")

    n_chunks = s // chunk

    # DVE/GpSimd/ACT work in "groups" of `group` columns (fewer instructions,
    # fewer semaphores); the PE still tiles matmuls at `chunk`<=512 (PSUM bank).
    group = min(2 * chunk, s)
    n_groups = s // group
    mm_per_group = group // chunk

    with tile.TileContext(nc) as tc:
        with ExitStack() as ctx:
            consts = ctx.enter_context(tc.tile_pool(name="consts", bufs=1))
            xpool = ctx.enter_context(tc.tile_pool(name="xb", bufs=1))
            opool = ctx.enter_context(tc.tile_pool(name="outs", bufs=2))
            shpool = ctx.enter_context(tc.tile_pool(name="sh", bufs=2))
            cpool = ctx.enter_context(tc.tile_pool(name="cc", bufs=2))
            pt_pool = ctx.enter_context(tc.tile_pool(name="pt", bufs=2, space="PSUM"))
            pl_pool = ctx.enter_context(tc.tile_pool(name="pl", bufs=2, space="PSUM"))

            # --- one-time constants first: w ahead of everything (the
            # softmax -> W1 chain is on the critical path), then batch 0's
            # first half so group-0 compute can start ASAP. ident goes via
            # the Scalar queue and A via GpSimd to keep Sync free.
            w_sb = consts.tile([F, F], f32)
            nc.sync.dma_start(out=w_sb, in_=w[:, :])
            ident_sb = consts.tile([F, F], f32)
            nc.scalar.dma_start(out=ident_sb, in_=ident_dr[:, :])
            a_sb = consts.tile([F, F], f32r)
            nc.gpsimd.dma_start(out=a_sb, in_=a_dr[:, :])

            # --- prefetch all batch inputs (whole shard fits in SBUF);
            # batch 0 split so its first stripe lands early
            xbs = []
            for bi in range(b_per):
                xb = xpool.tile([F, s + 2], f32, tag=f"xb{bi}")
                nc.gpsimd.memset(xb[:, 0:1], 0.0)
                nc.gpsimd.memset(xb[:, s + 1 : s + 2], 0.0)
                if bi == 0:
                    hs = group + 2
                    nc.sync.dma_start(out=xb[:, 1:hs], in_=xt[bi, :, 0 : hs - 1])
                    nc.sync.dma_start(
                        out=xb[:, hs : s + 1], in_=xt[bi, :, hs - 1 : s]
                    )
                else:
                    nc.sync.dma_start(out=xb[:, 1 : s + 1], in_=xt[bi, :, :])
                xbs.append(xb)

            # --- softmax(w) -> W1 = -0.5 I - 0.5 w_sm.T
            negmax = consts.tile([F, 1], f32)
            nc.vector.tensor_reduce(
                negmax, w_sb, axis=mybir.AxisListType.X, op=Alu.max, negate=True
            )
            e_sb = consts.tile([F, F], f32)
            nc.scalar.activation(e_sb, w_sb, Act.Exp, bias=negmax[:, 0:1], scale=1.0)
            ssum = consts.tile([F, 1], f32)
            nc.vector.tensor_reduce(ssum, e_sb, axis=mybir.AxisListType.X, op=Alu.add)
            rinv = consts.tile([F, 1], f32)
            nc.vector.reciprocal(rinv, ssum)
            # h = -0.5 * w_sm  (rowwise e * rinv, then * -0.5)
            h_sb = consts.tile([F, F], f32)
            nc.vector.tensor_scalar(h_sb, e_sb, rinv[:, 0:1], -0.5, Alu.mult, Alu.mult)
            ht_ps = pt_pool.tile([F, F], f32, tag="pt")
            nc.tensor.transpose(ht_ps, h_sb, ident_sb)
            w1_sb = consts.tile([F, F], f32)
            nc.vector.scalar_tensor_tensor(
                w1_sb, ident_sb, -0.5, ht_ps, Alu.mult, Alu.add
            )

            # --- main loop: per batch, per `group`-column stripe.
            # Emission order keeps the PE queue dense: all tmdo matmuls of
            # the batch (deps: xb + W1 only) ahead of the lap matmuls
            # (deps: DVE-produced c), so the PE never gap-stalls on the
            # slower DVE/GpSimd feeds (HAM stays warm).
            for bi in range(b_per):
                xb = xbs[bi]
                out_t = opool.tile([F, s], f32)
                out_l = opool.tile([F, s], f32)

                shs, cs_, pts, pls = [], [], [], []
                for g in range(n_groups):
                    g0 = 1 + g * group
                    sh = shpool.tile([F, group], f32, tag=f"sh{g % 2}")
                    nc.gpsimd.tensor_add(
                        sh, xb[:, g0 - 1 : g0 - 1 + group], xb[:, g0 + 1 : g0 + 1 + group]
                    )
                    # c = sh - 2x in f32r; lap = A @ c at full fp32r matmul
                    # rate. lap weights are exact small ints, f32r's ~1e-4
                    # rel error is well inside tolerance.
                    c_sb = cpool.tile([F, group], f32r, tag=f"cc{g % 2}")
                    nc.vector.scalar_tensor_tensor(
                        c_sb, xb[:, g0 : g0 + group], -2.0, sh, Alu.mult, Alu.add
                    )
                    shs.append(sh)
                    cs_.append(c_sb)

                for g in range(n_groups):
                    g0 = 1 + g * group
                    pt = pt_pool.tile([F, group], f32, tag="pt")
                    for m in range(mm_per_group):
                        ms = slice(m * chunk, (m + 1) * chunk)
                        nc.tensor.matmul(
                            pt[:, ms],
                            w1_sb,
                            xb[:, g0 + m * chunk : g0 + (m + 1) * chunk],
                            start=True,
                            stop=True,
                        )
                    pts.append(pt)
                for g in range(n_groups):
                    pl = pl_pool.tile([F, group], f32)
                    for m in range(mm_per_group):
                        ms = slice(m * chunk, (m + 1) * chunk)
                        nc.tensor.matmul(
                            pl[:, ms], a_sb, cs_[g][:, ms], start=True, stop=True
                        )
                    pls.append(pl)

                for g in range(n_groups):
                    gs = slice(g * group, (g + 1) * group)
                    pt, pl, sh = pts[g], pls[g], shs[g]
                    nc.vector.scalar_tensor_tensor(
                        out_t[:, gs], sh, 0.5, pt, Alu.mult, Alu.add
                    )
                    # time_diff is 0 at the batch's seq boundaries: there
                    # tmdo = 0.5*(x - y) = psum_t + x.
                    if g == 0:
                        nc.vector.tensor_add(out_t[:, 0:1], pt[:, 0:1], xb[:, 1:2])
                    if g == n_groups - 1:
                        nc.vector.tensor_add(
                            out_t[:, s - 1 : s], pt[:, group - 1 : group], xb[:, s : s + 1]
                        )
                    nc.scalar.copy(out_l[:, gs], pl)
                    nc.sync.dma_start(out=tmdo_d[bi, :, gs], in_=out_t[:, gs])
                    nc.sync.dma_start(out=lap_d[bi, :, gs], in_=out_l[:, gs])

    nc.compile()
    return nc


def _get_nc():
    if "nc" not in _NC_CACHE:
        _NC_CACHE["nc"] = _build_nc()
    return _NC_CACHE["nc"]


def run_kernel_raw(x, weight_matrix, **run_kwargs):
    """Returns (BassKernelResults, tmdo, lap). run_kwargs forwarded to
    run_bass_kernel_spmd (e.g. trace=True)."""
    from concourse.bass_utils import run_bass_kernel_spmd

    x = np.ascontiguousarray(np.asarray(x, dtype=np.float32))
    w = np.ascontiguousarray(np.asarray(weight_matrix, dtype=np.float32))

    nc = _get_nc()
    xs = x.reshape(N_CORES, B_PER, S, F)
    in_maps = [
        {"xt": np.ascontiguousarray(xs[c].transpose(0, 2, 1)), "w": w}
        for c in range(N_CORES)
    ]
    br = run_bass_kernel_spmd(nc, in_maps, core_ids=list(range(N_CORES)), **run_kwargs)
    res = br.results

    tmdo = np.empty((B, S, F), np.float32)
    lap = np.empty((B, S, F), np.float32)
    for c in range(N_CORES):
        tmdo[c * B_PER : (c + 1) * B_PER] = res[c]["tmdo_t"].transpose(0, 2, 1)
        lap[c * B_PER : (c + 1) * B_PER] = res[c]["lap_t"].transpose(0, 2, 1)
    return br, tmdo, lap


def kernel(x, weight_matrix):
    _, tmdo, lap = run_kernel_raw(x, weight_matrix)
    return tmdo, lap


# revision 12
# speedup vs baseline: 1.0642x; 1.0642x over previous
"""Trainium2 Bass kernel for EnhancedTMDO.

Computes, for x [B, S, F] and weight_matrix [F, F]:
  tmdo = 0.5 * time_diff + 0.5 * (x - x @ softmax(w, axis=1).T)
  lap  = 3x3 Laplacian-style conv over the (S, F) plane, SAME zero padding

Strategy (8 NeuronCores, data-parallel over batch, 4 batches per core):
  * All device compute happens in transposed layout [F=128 partitions, S free].
    The host pre-transposes each core's shard ([4,2048,128] -> [4,128,2048])
    and post-transposes the outputs back. This puts the feature contraction
    (the 128x128 matmuls) on the partition axis, and turns all seq-direction
    stencils into cheap shifted-AP reads along the free axis.
  * The 3x3 conv kernel is separable: outer([1,-2,1],[1,-2,1]). The seq-dim
    1D conv is computed from shifted adds; the feature-dim 1D conv is a
    tridiagonal 128x128 matmul on the TensorEngine.
  * Per 512-column chunk:
      psum_t = (-0.5 I - 0.5 w_sm.T).T @ x          (TensorE)
      sh     = x[s-1] + x[s+1]                      (VectorE, shifted APs)
      psum_l = A @ sh + (-2A) @ x                   (TensorE, accumulating;
                                                     A = tridiag(1,-2,1))
      tmdo   = 0.5*sh + psum_t                      (VectorE fused op)
      lap    = copy(psum_l)                         (ScalarE)
    plus a 1-column fixup per batch edge (time_diff is zero at s boundaries
    while the conv uses zero padding).
"""

from contextlib import ExitStack

import numpy as np

N_CORES = 8
B, S, F = 32, 2048, 128
B_PER = B // N_CORES
CHUNK = 512

_NC_CACHE = {}


def _build_nc(b_per=B_PER, s=S, chunk=CHUNK):
    import concourse.bacc as bacc
    import concourse.tile as tile
    from concourse import mybir

    f32 = mybir.dt.float32
    f32r = mybir.dt.float32r
    Alu = mybir.AluOpType
    Act = mybir.ActivationFunctionType

    nc = bacc.Bacc(None, target_bir_lowering=False)

    xt = nc.declare_dram_parameter("xt", [b_per, F, s], f32, isOutput=False)
    w = nc.declare_dram_parameter("w", [F, F], f32, isOutput=False)
    tmdo_d = nc.declare_dram_parameter("tmdo_t", [b_per, F, s], f32, isOutput=True)
    lap_d = nc.declare_dram_parameter("lap_t", [b_per, F, s], f32, isOutput=True)

    ident_np = np.eye(F, dtype=np.float32)
    a_np = (
        np.diag(np.full(F, -2.0))
        + np.diag(np.ones(F - 1), 1)
        + np.diag(np.ones(F - 1), -1)
    ).astype(np.float32)
    ident_dr = nc.inline_tensor(ident_np, "ident")
    a_dr = nc.inline_tensor(a_np, "amat")

    n_chunks = s // chunk

    # DVE/GpSimd/ACT work in "groups" of `group` columns (fewer instructions,
    # fewer semaphores); the PE still tiles matmuls at `chunk`<=512 (PSUM bank).
    group = min(2 * chunk, s)
    n_groups = s // group
    mm_per_group = group // chunk

    with tile.TileContext(nc) as tc:
        with ExitStack() as ctx:
            consts = ctx.enter_context(tc.tile_pool(name="consts", bufs=1))
            xpool = ctx.enter_context(tc.tile_pool(name="xb", bufs=1))
            opool = ctx.enter_context(tc.tile_pool(name="outs", bufs=2))
            shpool = ctx.enter_context(tc.tile_pool(name="sh", bufs=2))
            cpool = ctx.enter_context(tc.tile_pool(name="cc", bufs=2))
            pt_pool = ctx.enter_context(tc.tile_pool(name="pt", bufs=2, space="PSUM"))
            pl_pool = ctx.enter_context(tc.tile_pool(name="pl", bufs=2, space="PSUM"))

            # --- one-time constants first: w ahead of everything (the
            # softmax -> W1 chain is on the critical path), then batch 0's
            # first half so group-0 compute can start ASAP. ident goes via
            # the Scalar queue and A via GpSimd to keep Sync free.
            w_sb = consts.tile([F, F], f32)
            nc.sync.dma_start(out=w_sb, in_=w[:, :])
            ident_sb = consts.tile([F, F], f32)
            nc.scalar.dma_start(out=ident_sb, in_=ident_dr[:, :])
            a_sb = consts.tile([F, F], f32r)
            nc.gpsimd.dma_start(out=a_sb, in_=a_dr[:, :])

            # --- prefetch all batch inputs (whole shard fits in SBUF);
            # batch 0 split so its first stripe lands early
            xbs = []
            for bi in range(b_per):
                xb = xpool.tile([F, s + 2], f32, tag=f"xb{bi}")
                nc.gpsimd.memset(xb[:, 0:1], 0.0)
                nc.gpsimd.memset(xb[:, s + 1 : s + 2], 0.0)
                if bi == 0:
                    hs = group + 2
                    nc.sync.dma_start(out=xb[:, 1:hs], in_=xt[bi, :, 0 : hs - 1])
                    nc.sync.dma_start(
                        out=xb[:, hs : s + 1], in_=xt[bi, :, hs - 1 : s]
                    )
                else:
                    nc.sync.dma_start(out=xb[:, 1 : s + 1], in_=xt[bi, :, :])
                xbs.append(xb)

            # --- softmax(w) -> W1 = -0.5 I - 0.5 w_sm.T
            # high_priority so the scheduler doesn't slot batch work ahead
            # of this chain (it gates every tmdo matmul).
            with tc.high_priority():
                negmax = consts.tile([F, 1], f32)
                nc.vector.tensor_reduce(
                    negmax, w_sb, axis=mybir.AxisListType.X, op=Alu.max, negate=True
                )
                e_sb = consts.tile([F, F], f32)
                nc.scalar.activation(
                    e_sb, w_sb, Act.Exp, bias=negmax[:, 0:1], scale=1.0
                )
                ssum = consts.tile([F, 1], f32)
                nc.vector.tensor_reduce(
                    ssum, e_sb, axis=mybir.AxisListType.X, op=Alu.add
                )
                rinv = consts.tile([F, 1], f32)
                nc.vector.reciprocal(rinv, ssum)
                # h = -0.5 * w_sm  (rowwise e * rinv, then * -0.5)
                h_sb = consts.tile([F, F], f32)
                nc.vector.tensor_scalar(
                    h_sb, e_sb, rinv[:, 0:1], -0.5, Alu.mult, Alu.mult
                )
                ht_ps = pt_pool.tile([F, F], f32, tag="pt")
                nc.tensor.transpose(ht_ps, h_sb, ident_sb)
                w1_sb = consts.tile([F, F], f32)
                nc.vector.scalar_tensor_tensor(
                    w1_sb, ident_sb, -0.5, ht_ps, Alu.mult, Alu.add
                )

            # --- main loop: per batch, per `group`-column stripe.
            # Emission order keeps the PE queue dense: all tmdo matmuls of
            # the batch (deps: xb + W1 only) ahead of the lap matmuls
            # (deps: DVE-produced c), so the PE never gap-stalls on the
            # slower DVE/GpSimd feeds (HAM stays warm).
            for bi in range(b_per):
                xb = xbs[bi]
                out_t = opool.tile([F, s], f32)
                out_l = opool.tile([F, s], f32)

                # sh and c computed batch-wide on DVE: the sh -> c and
                # sh -> tmdo deps become same-engine (no semaphores).
                sh = shpool.tile([F, s], f32)
                nc.vector.tensor_add(sh, xb[:, 0:s], xb[:, 2 : s + 2])
                # c = sh - 2x in f32r; lap = A @ c at full fp32r matmul
                # rate. lap weights are exact small ints, f32r's ~1e-4
                # rel error is well inside tolerance.
                c_sb = cpool.tile([F, s], f32r)
                nc.vector.scalar_tensor_tensor(
                    c_sb, xb[:, 1 : s + 1], -2.0, sh, Alu.mult, Alu.add
                )

                pts, pls = [], []
                for g in range(n_groups):
                    g0 = 1 + g * group
                    pt = pt_pool.tile([F, group], f32, tag="pt")
                    for m in range(mm_per_group):
                        ms = slice(m * chunk, (m + 1) * chunk)
                        nc.tensor.matmul(
                            pt[:, ms],
                            w1_sb,
                            xb[:, g0 + m * chunk : g0 + (m + 1) * chunk],
                            start=True,
                            stop=True,
                        )
                    pts.append(pt)
                for g in range(n_groups):
                    pl = pl_pool.tile([F, group], f32)
                    for m in range(mm_per_group):
                        ms = slice(m * chunk, (m + 1) * chunk)
                        cm = slice(g * group + m * chunk, g * group + (m + 1) * chunk)
                        nc.tensor.matmul(
                            pl[:, ms], a_sb, c_sb[:, cm], start=True, stop=True
                        )
                    pls.append(pl)

                for g in range(n_groups):
                    gs = slice(g * group, (g + 1) * group)
                    pt, pl = pts[g], pls[g]
                    nc.vector.scalar_tensor_tensor(
                        out_t[:, gs], sh[:, gs], 0.5, pt, Alu.mult, Alu.add
                    )
                    # time_diff is 0 at the batch's seq boundaries: there
                    # tmdo = 0.5*(x - y) = psum_t + x.
                    if g == 0:
                        nc.vector.tensor_add(out_t[:, 0:1], pt[:, 0:1], xb[:, 1:2])
                    if g == n_groups - 1:
                        nc.vector.tensor_add(
                            out_t[:, s - 1 : s], pt[:, group - 1 : group], xb[:, s : s + 1]
                        )
                    nc.scalar.copy(out_l[:, gs], pl)
                    nc.sync.dma_start(out=tmdo_d[bi, :, gs], in_=out_t[:, gs])
                    nc.sync.dma_start(out=lap_d[bi, :, gs], in_=out_l[:, gs])

    nc.compile()
    return nc


def _get_nc():
    if "nc" not in _NC_CACHE:
        _NC_CACHE["nc"] = _build_nc()
    return _NC_CACHE["nc"]


def run_kernel_raw(x, weight_matrix, **run_kwargs):
    """Returns (BassKernelResults, tmdo, lap). run_kwargs forwarded to
    run_bass_kernel_spmd (e.g. trace=True)."""
    from concourse.bass_utils import run_bass_kernel_spmd

    x = np.ascontiguousarray(np.asarray(x, dtype=np.float32))
    w = np.ascontiguousarray(np.asarray(weight_matrix, dtype=np.float32))

    nc = _get_nc()
    xs = x.reshape(N_CORES, B_PER, S, F)
    in_maps = [
        {"xt": np.ascontiguousarray(xs[c].transpose(0, 2, 1)), "w": w}
        for c in range(N_CORES)
    ]
    br = run_bass_kernel_spmd(nc, in_maps, core_ids=list(range(N_CORES)), **run_kwargs)
    res = br.results

    tmdo = np.empty((B, S, F), np.float32)
    lap = np.empty((B, S, F), np.float32)
    for c in range(N_CORES):
        tmdo[c * B_PER : (c + 1) * B_PER] = res[c]["tmdo_t"].transpose(0, 2, 1)
        lap[c * B_PER : (c + 1) * B_PER] = res[c]["lap_t"].transpose(0, 2, 1)
    return br, tmdo, lap


def kernel(x, weight_matrix):
    _, tmdo, lap = run_kernel_raw(x, weight_matrix)
    return tmdo, lap


# revision 16
# speedup vs baseline: 1.1817x; 1.1104x over previous
"""Trainium2 Bass kernel for EnhancedTMDO.

Computes, for x [B, S, F] and weight_matrix [F, F]:
  tmdo = 0.5 * time_diff + 0.5 * (x - x @ softmax(w, axis=1).T)
  lap  = 3x3 Laplacian-style conv over the (S, F) plane, SAME zero padding

Strategy (8 NeuronCores, data-parallel over batch, 4 batches per core):
  * All device compute happens in transposed layout [F=128 partitions, S free].
    The host pre-transposes each core's shard ([4,2048,128] -> [4,128,2048])
    and post-transposes the outputs back. This puts the feature contraction
    (the 128x128 matmuls) on the partition axis, and turns all seq-direction
    stencils into cheap shifted-AP reads along the free axis.
  * The 3x3 conv kernel is separable: outer([1,-2,1],[1,-2,1]). The seq-dim
    1D conv comes from one shifted add (sh = x<<1 + x>>1); the feature-dim
    conv is a tridiagonal 128x128 matmul.
  * Everything runs in float32r (TF32-like single-pass matmuls, 4x faster
    than fp32's two half-speed passes). Measured error ~1.6e-4 relative on
    random 128-contractions; here the tmdo weights have row norms ~0.55 so
    the observed error stays ~1e-4 of output scale.
  * Per 512-column chunk, with PSUM accumulation (interior columns):
      pt = W1 @ x + (0.5 I) @ sh     W1 = -0.5(I + w_sm.T)   -> tmdo
      pl = A @ sh + (-2A) @ x        A  = tridiag(1,-2,1)    -> lap
    ScalarE copies PSUM -> SBUF, Sync DMAs out. Batch seq-boundary columns
    get a 2-op DVE fixup (time_diff is zero there, not zero-padded).
"""

from contextlib import ExitStack

import numpy as np

N_CORES = 8
B, S, F = 32, 2048, 128
B_PER = B // N_CORES
CHUNK = 512

_NC_CACHE = {}


def _build_nc(b_per=B_PER, s=S, chunk=CHUNK):
    import concourse.bacc as bacc
    import concourse.tile as tile
    from concourse import mybir

    f32 = mybir.dt.float32
    f32r = mybir.dt.float32r
    Alu = mybir.AluOpType
    Act = mybir.ActivationFunctionType

    nc = bacc.Bacc(None, target_bir_lowering=False)

    # xt arrives host-padded with one zero column on each side of every
    # batch ([F, s+2]) — the SAME-padding halo for the seq-dim stencils.
    xt = nc.declare_dram_parameter("xt", [b_per, F, s + 2], f32r, isOutput=False)
    w = nc.declare_dram_parameter("w", [F, F], f32, isOutput=False)
    tmdo_d = nc.declare_dram_parameter("tmdo_t", [b_per, F, s], f32, isOutput=True)
    lap_d = nc.declare_dram_parameter("lap_t", [b_per, F, s], f32, isOutput=True)

    ident_np = np.eye(F, dtype=np.float32)
    a_np = (
        np.diag(np.full(F, -2.0))
        + np.diag(np.ones(F - 1), 1)
        + np.diag(np.ones(F - 1), -1)
    ).astype(np.float32)
    ident_dr = nc.inline_tensor(ident_np, "ident")
    a_dr = nc.inline_tensor(a_np, "amat")
    m2a_dr = nc.inline_tensor((-2.0 * a_np).astype(np.float32), "m2amat")
    halfi_dr = nc.inline_tensor((0.5 * ident_np).astype(np.float32), "halfi")

    group = min(2 * chunk, s)
    n_groups = s // group
    mm_per_group = group // chunk

    with tile.TileContext(nc) as tc:
        with ExitStack() as ctx:
            consts = ctx.enter_context(tc.tile_pool(name="consts", bufs=1))
            xpool = ctx.enter_context(tc.tile_pool(name="xb", bufs=1))
            opool = ctx.enter_context(tc.tile_pool(name="outs", bufs=2))
            shpool = ctx.enter_context(tc.tile_pool(name="sh", bufs=2))
            pt_pool = ctx.enter_context(tc.tile_pool(name="pt", bufs=2, space="PSUM"))
            pl_pool = ctx.enter_context(tc.tile_pool(name="pl", bufs=2, space="PSUM"))

            # --- one-time constants first: w ahead of everything (the
            # softmax -> W1 chain gates the tmdo matmuls). ident goes via
            # the Scalar queue and the f32r constants via GpSimd cast-DMAs
            # to keep the Sync queue free for the bulk loads.
            w_sb = consts.tile([F, F], f32)
            nc.sync.dma_start(out=w_sb, in_=w[:, :])
            ident_sb = consts.tile([F, F], f32)
            nc.scalar.dma_start(out=ident_sb, in_=ident_dr[:, :])
            a_sb = consts.tile([F, F], f32r)
            nc.gpsimd.dma_start(out=a_sb, in_=a_dr[:, :])
            m2a_sb = consts.tile([F, F], f32r)
            nc.gpsimd.dma_start(out=m2a_sb, in_=m2a_dr[:, :])
            halfi_sb = consts.tile([F, F], f32r)
            nc.gpsimd.dma_start(out=halfi_sb, in_=halfi_dr[:, :])

            # --- prefetch all batch inputs (whole shard fits in SBUF);
            # batch 0 split so its first stripe lands early
            xbs = []
            for bi in range(b_per):
                xb = xpool.tile([F, s + 2], f32r, tag=f"xb{bi}")
                if bi == 0:
                    hs = group + 2
                    nc.sync.dma_start(out=xb[:, 0:hs], in_=xt[bi, :, 0:hs])
                    nc.sync.dma_start(
                        out=xb[:, hs : s + 2], in_=xt[bi, :, hs : s + 2]
                    )
                else:
                    nc.sync.dma_start(out=xb, in_=xt[bi, :, :])
                xbs.append(xb)

            # --- softmax(w) -> W1 = -0.5 I - 0.5 w_sm.T (f32r), at high
            # priority so the scheduler doesn't slot batch work ahead of it.
            with tc.high_priority():
                negmax = consts.tile([F, 1], f32)
                nc.vector.tensor_reduce(
                    negmax, w_sb, axis=mybir.AxisListType.X, op=Alu.max, negate=True
                )
                e_sb = consts.tile([F, F], f32)
                nc.scalar.activation(
                    e_sb, w_sb, Act.Exp, bias=negmax[:, 0:1], scale=1.0
                )
                ssum = consts.tile([F, 1], f32)
                nc.vector.tensor_reduce(
                    ssum, e_sb, axis=mybir.AxisListType.X, op=Alu.add
                )
                rinv = consts.tile([F, 1], f32)
                nc.vector.reciprocal(rinv, ssum)
                # h = -0.5 * w_sm  (rowwise e * rinv, then * -0.5)
                h_sb = consts.tile([F, F], f32)
                nc.vector.tensor_scalar(
                    h_sb, e_sb, rinv[:, 0:1], -0.5, Alu.mult, Alu.mult
                )
                ht_ps = pt_pool.tile([F, F], f32, tag="pt")
                nc.tensor.transpose(ht_ps, h_sb, ident_sb)
                w1_sb = consts.tile([F, F], f32r)
                nc.vector.scalar_tensor_tensor(
                    w1_sb, ident_sb, -0.5, ht_ps, Alu.mult, Alu.add
                )

            # --- main loop
            for bi in range(b_per):
                xb = xbs[bi]
                out_t = opool.tile([F, s], f32)
                out_l = opool.tile([F, s], f32)

                # sh = x[s-1] + x[s+1] (f32r), batch-wide in one DVE op
                sh = shpool.tile([F, s], f32r)
                nc.vector.tensor_add(sh, xb[:, 0:s], xb[:, 2 : s + 2])

                pts, pls = [], []
                for g in range(n_groups):
                    g0 = 1 + g * group
                    pt = pt_pool.tile([F, group], f32, tag="pt")
                    for m in range(mm_per_group):
                        ms = slice(m * chunk, (m + 1) * chunk)
                        xs = slice(g0 + m * chunk, g0 + (m + 1) * chunk)
                        ss = slice(g * group + m * chunk, g * group + (m + 1) * chunk)
                        nc.tensor.matmul(
                            pt[:, ms], w1_sb, xb[:, xs], start=True, stop=False
                        )
                        nc.tensor.matmul(
                            pt[:, ms], halfi_sb, sh[:, ss], start=False, stop=True
                        )
                    pts.append(pt)
                for g in range(n_groups):
                    g0 = 1 + g * group
                    pl = pl_pool.tile([F, group], f32)
                    for m in range(mm_per_group):
                        ms = slice(m * chunk, (m + 1) * chunk)
                        xs = slice(g0 + m * chunk, g0 + (m + 1) * chunk)
                        ss = slice(g * group + m * chunk, g * group + (m + 1) * chunk)
                        nc.tensor.matmul(
                            pl[:, ms], a_sb, sh[:, ss], start=True, stop=False
                        )
                        nc.tensor.matmul(
                            pl[:, ms], m2a_sb, xb[:, xs], start=False, stop=True
                        )
                    pls.append(pl)

                for g in range(n_groups):
                    gs = slice(g * group, (g + 1) * group)
                    pt, pl = pts[g], pls[g]
                    nc.scalar.copy(out_t[:, gs], pt)
                    # time_diff is 0 at the batch's seq boundaries: there
                    # tmdo = 0.5*(x - y) = (pt - 0.5*sh) + x.
                    if g == 0:
                        nc.vector.scalar_tensor_tensor(
                            out_t[:, 0:1], sh[:, 0:1], -0.5, pt[:, 0:1],
                            Alu.mult, Alu.add,
                        )
                        nc.vector.tensor_add(out_t[:, 0:1], out_t[:, 0:1], xb[:, 1:2])
                    if g == n_groups - 1:
                        nc.vector.scalar_tensor_tensor(
                            out_t[:, s - 1 : s],
                            sh[:, s - 1 : s], -0.5, pt[:, group - 1 : group],
                            Alu.mult, Alu.add,
                        )
                        nc.vector.tensor_add(
                            out_t[:, s - 1 : s], out_t[:, s - 1 : s], xb[:, s : s + 1]
                        )
                    nc.scalar.copy(out_l[:, gs], pl)
                    nc.sync.dma_start(out=tmdo_d[bi, :, gs], in_=out_t[:, gs])
                    nc.sync.dma_start(out=lap_d[bi, :, gs], in_=out_l[:, gs])

    nc.compile()
    return nc


def _get_nc():
    if "nc" not in _NC_CACHE:
        _NC_CACHE["nc"] = _build_nc()
    return _NC_CACHE["nc"]


def run_kernel_raw(x, weight_matrix, **run_kwargs):
    """Returns (BassKernelResults, tmdo, lap). run_kwargs forwarded to
    run_bass_kernel_spmd (e.g. trace=True)."""
    from concourse.bass_utils import run_bass_kernel_spmd

    x = np.ascontiguousarray(np.asarray(x, dtype=np.float32))
    w = np.ascontiguousarray(np.asarray(weight_matrix, dtype=np.float32))

    nc = _get_nc()
    xs = x.reshape(N_CORES, B_PER, S, F)
    xt_all = np.zeros((N_CORES, B_PER, F, S + 2), np.float32)
    xt_all[:, :, :, 1 : S + 1] = xs.transpose(0, 1, 3, 2)
    in_maps = [{"xt": xt_all[c], "w": w} for c in range(N_CORES)]
    br = run_bass_kernel_spmd(nc, in_maps, core_ids=list(range(N_CORES)), **run_kwargs)
    res = br.results

    tmdo = np.empty((B, S, F), np.float32)
    lap = np.empty((B, S, F), np.float32)
    for c in range(N_CORES):
        tmdo[c * B_PER : (c + 1) * B_PER] = res[c]["tmdo_t"].transpose(0, 2, 1)
        lap[c * B_PER : (c + 1) * B_PER] = res[c]["lap_t"].transpose(0, 2, 1)
    return br, tmdo, lap


def kernel(x, weight_matrix):
    _, tmdo, lap = run_kernel_raw(x, weight_matrix)
    return tmdo, lap
